# revision 1
# baseline (speedup 1.0000x reference)
"""Trainium2 Bass kernel for AttentionFact:
    scores = einsum('bsh,ch->bcs', hidden, querys)
    factor = softmax(scores, axis=2)
    out    = einsum('bcs,bsh->bch', factor, hidden).reshape(B, C*H)

Shapes: B=16, S=4096, H=1024, C=64, fp32.

Strategy: data-parallel over batch. Each of the 8 NeuronCores handles 2
batches; querys (small) is replicated, pre-transposed on host into
[128, 8, 64] h-chunk layout. No collectives: host concatenates the
per-core [2, C*H] outputs.

Per-core dataflow (per batch):
  - stream hidden in 8 s-tiles of 512 rows with f32->fp16 cast during
    the DMA (SWDGE); natural-layout fp16 tiles [s=128p x (q,h)] stay
    resident in SBUF for the second einsum
  - produce hT [h=128p x s] blocks for the scores matmul (contract over
    h): most via PE-transpose + PSUM evacuation, a tunable fraction via
    the DMA xbar transpose (2-byte dtype) to balance PE vs DMA load
  - scores[c, s] accumulated in f32 PSUM over the 8 h-chunks, qT
    stationary (fp16 -> fast weight loads)
  - two-pass softmax over s in f32: DVE reduce_max (negated), ScalarE
    exp(x - max) writing fp16 with fused f32 row-sum accumulation
  - PE-transpose factor blocks -> factorT [s=128p x c] fp16; second
    matmul (contract over s) with the resident fp16 hidden tiles
  - normalize by 1/rowsum during PSUM->SBUF evacuation (f32); DMA out

The two batches are phase-interleaved (p1(b0), sm(b0), p1(b1), p3(b0),
sm(b1), p3(b1)) so the PE never idles across the softmax barrier.
"""

import numpy as np

import concourse.bass as bass
import concourse.mybir as mybir
import concourse.tile as tile
from concourse import bacc
from concourse.bass_utils import run_bass_kernel_spmd

B, S, H, C = 16, 4096, 1024, 64
NCORES = 8
BPC = B // NCORES          # batches per core
ST = 8                     # s-tiles per batch (512 rows each)
SQ = 4                     # 128-row subtiles per s-tile
HJ = H // 128              # h-chunks (8)

F32 = mybir.dt.float32
F16 = mybir.dt.float16

DMA_T_JPS = set()            # jp values whose hT blocks go via DMA xbar transpose


def build_nc():
    nc = bacc.Bacc("TRN2", target_bir_lowering=False, debug=False)
    hidden = nc.declare_dram_parameter("hidden", [BPC, S, H], F32, isOutput=False)
    qT = nc.declare_dram_parameter("qT", [128, HJ, C], F16, isOutput=False)
    ident = nc.declare_dram_parameter("ident", [128, 128], F16, isOutput=False)
    out = nc.declare_dram_parameter("out", [BPC, C, H], F32, isOutput=True)

    with tile.TileContext(nc) as tc:
        with (
            tc.tile_pool(name="const", bufs=1) as const_pool,
            tc.tile_pool(name="nat", bufs=15) as nat_pool,
            tc.tile_pool(name="hT", bufs=4) as hT_pool,
            tc.tile_pool(name="scores", bufs=2) as scores_pool,
            tc.tile_pool(name="expf", bufs=2) as exp_pool,
            tc.tile_pool(name="fT", bufs=2) as fT_pool,
            tc.tile_pool(name="stats", bufs=4) as stats_pool,
            tc.tile_pool(name="outp", bufs=2) as out_pool,
            tc.tile_pool(name="psT", bufs=2, space="PSUM") as psT_pool,
            tc.tile_pool(name="psS", bufs=2, space="PSUM") as psS_pool,
            tc.tile_pool(name="psF", bufs=2, space="PSUM") as psF_pool,
            tc.tile_pool(name="psR", bufs=1, space="PSUM") as psR_pool,
        ):
            ident_sb = const_pool.tile([128, 128], F16, tag="ident")
            nc.sync.dma_start(out=ident_sb[:], in_=ident[:])
            qT_sb = const_pool.tile([128, HJ, C], F16, tag="qT")
            nc.sync.dma_start(out=qT_sb[:], in_=qT[:])

            nat_tiles = {}
            scores_tiles = {}
            exp_tiles = {}
            rinv_tiles = {}

            def phase1(b):
                scores_sb = scores_pool.tile([C, S], F32, tag="scores")
                scores_tiles[b] = scores_sb
                for st in range(ST):
                    nat_t = nat_pool.tile([128, SQ, H], F16, tag="nat")
                    nat_tiles[(b, st)] = nat_t
                    src = hidden[b, st * 512:(st + 1) * 512, :].rearrange(
                        "(q p) h -> p q h", p=128
                    )
                    nc.gpsimd.dma_start(out=nat_t[:], in_=src)

                    ps_sc = psS_pool.tile([C, 512], F32, tag="psS")
                    for jp in range(HJ // 2):  # pairs of h-chunks
                        hT = hT_pool.tile([128, 1024], F16, tag="hT")
                        if jp in DMA_T_JPS:
                            for ji in range(2):
                                j = jp * 2 + ji
                                for q in range(SQ):
                                    nc.sync.dma_start_transpose(
                                        hT[:, ji * 512 + q * 128:
                                           ji * 512 + (q + 1) * 128],
                                        nat_t[:, q, j * 128:(j + 1) * 128],
                                    )
                        else:
                            ps_t = psT_pool.tile([128, 1024], F16, tag="psT")
                            for ji in range(2):
                                j = jp * 2 + ji
                                for q in range(SQ):
                                    nc.tensor.transpose(
                                        ps_t[:, ji * 512 + q * 128:
                                             ji * 512 + (q + 1) * 128],
                                        nat_t[:, q, j * 128:(j + 1) * 128],
                                        ident_sb[:],
                                    )
                            if jp % 2 == 0:
                                nc.scalar.copy(hT[:], ps_t[:])
                            else:
                                nc.vector.tensor_copy(hT[:], ps_t[:])
                        for ji in range(2):
                            j = jp * 2 + ji
                            nc.tensor.matmul(
                                ps_sc[:],
                                qT_sb[:, j, :],
                                hT[:, ji * 512:(ji + 1) * 512],
                                start=(j == 0),
                                stop=(j == HJ - 1),
                            )
                    nc.vector.tensor_copy(
                        scores_sb[:, st * 512:(st + 1) * 512], ps_sc[:]
                    )

            def softmax(b):
                scores_sb = scores_tiles[b]
                negmax = stats_pool.tile([C, 1], F32, tag="negmax")
                nc.vector.reduce_max(
                    negmax[:], scores_sb[:], axis=mybir.AxisListType.X, negate=True
                )
                rowsum = stats_pool.tile([C, 1], F32, tag="rowsum")
                exp_sb = exp_pool.tile([C, S], F16, tag="expf")
                exp_tiles[b] = exp_sb
                nc.scalar.activation(
                    exp_sb[:],
                    scores_sb[:],
                    mybir.ActivationFunctionType.Exp,
                    bias=negmax[:],
                    accum_out=rowsum[:],
                )
                rinv = stats_pool.tile([C, 1], F32, tag="rinv")
                rinv_tiles[b] = rinv
                nc.vector.reciprocal(rinv[:], rowsum[:])

            def phase3(b):
                exp_sb = exp_tiles[b]
                ps_res = psR_pool.tile([C, H], F32, tag="psR")
                for kg in range(8):  # groups of 4 s-chunks of 128
                    ps_f = psF_pool.tile([128, 4 * C], F16, tag="psF")
                    for ki in range(4):
                        k = kg * 4 + ki
                        nc.tensor.transpose(
                            ps_f[:, ki * C:(ki + 1) * C],
                            exp_sb[:, k * 128:(k + 1) * 128],
                            ident_sb[:C, :C],
                        )
                    fT = fT_pool.tile([128, 4 * C], F16, tag="fT")
                    if kg % 2 == 0:
                        nc.scalar.copy(fT[:], ps_f[:])
                    else:
                        nc.vector.tensor_copy(fT[:], ps_f[:])
                    for ki in range(4):
                        k = kg * 4 + ki
                        st, q = divmod(k, SQ)
                        for h2 in range(2):
                            nc.tensor.matmul(
                                ps_res[:, h2 * 512:(h2 + 1) * 512],
                                fT[:, ki * C:(ki + 1) * C],
                                nat_tiles[(b, st)][:, q, h2 * 512:(h2 + 1) * 512],
                                start=(k == 0),
                                stop=(k == S // 128 - 1),
                            )
                out_sb = out_pool.tile([C, H], F32, tag="out")
                nc.vector.tensor_scalar_mul(out_sb[:], ps_res[:], rinv_tiles[b][:])
                nc.sync.dma_start(out=out[b], in_=out_sb[:])

            phase1(0)
            softmax(0)
            phase1(1)
            phase3(0)
            softmax(1)
            phase3(1)

    nc.compile()
    return nc


_NC_CACHE = None


def _get_nc():
    global _NC_CACHE
    if _NC_CACHE is None:
        _NC_CACHE = build_nc()
    return _NC_CACHE


def kernel(hidden, querys):
    hidden = np.ascontiguousarray(np.asarray(hidden), dtype=np.float32)
    querys = np.ascontiguousarray(np.asarray(querys), dtype=np.float32)
    assert hidden.shape == (B, S, H) and querys.shape == (C, H)

    # qT[k, j, c] = querys[c, j*128 + k]  (h-chunk-major transposed layout)
    qT = np.ascontiguousarray(
        querys.T.reshape(HJ, 128, C).transpose(1, 0, 2)
    ).astype(np.float16)
    ident = np.eye(128, dtype=np.float16)

    nc = _get_nc()
    in_maps = [
        {
            "hidden": np.ascontiguousarray(hidden[i * BPC:(i + 1) * BPC]),
            "qT": qT,
            "ident": ident,
        }
        for i in range(NCORES)
    ]
    res = run_bass_kernel_spmd(nc, in_maps, core_ids=list(range(NCORES)))
    global LAST_RESULTS
    LAST_RESULTS = res
    outs = [np.asarray(res.results[i]["out"]).reshape(BPC, C * H)
            for i in range(NCORES)]
    return np.concatenate(outs, axis=0)


LAST_RESULTS = None



# revision 16
# speedup vs baseline: 1.3263x; 1.3263x over previous
"""Trainium2 Bass kernel for AttentionFact:
    scores = einsum('bsh,ch->bcs', hidden, querys)
    factor = softmax(scores, axis=2)
    out    = einsum('bcs,bsh->bch', factor, hidden).reshape(B, C*H)

Shapes: B=16, S=4096, H=1024, C=64, fp32.

Strategy: data-parallel over batch. Each of the 8 NeuronCores handles 2
batches; querys (small) is replicated, pre-transposed on host into
[128, 8, 64] h-chunk layout. No collectives: host concatenates the
per-core [2, C*H] outputs.

v2 — PE column-tiling + fused softmax plumbing:
  - hidden streamed once per batch in 8 s-tiles of 512 rows with
    f32->fp16 cast during the DMA (SWDGE); fp16 natural-layout tiles
    stay resident in SBUF for the second einsum (nat bufs=16: the full
    16 MB working set is resident, the load queue never stalls)
  - hT blocks produced by PE transpose, evacuated PSUM->SBUF on DVE
    (3/4) + ScalarE (1/4)
  - mm1 is column-tiled over the two 256-col s-halves of each s-tile:
    both strips share the stationary qT[j] and stream their own hT
    columns concurrently in col-strips (0,0)/(0,64); M=64 would
    otherwise leave half the PE array idle
  - evacuation of each strip is one DVE tensor_tensor_reduce (in1 is an
    SBUF zeros tile: the verifier allows only one PSUM input): writes
    -(scores) into a [128, 2048] tile (c on partitions 0:64 for s <
    2048, 64:128 above) with a fused running min = -(running max)
  - softmax per s-half: ScalarE exp(scale=-1, bias=-max) with fused
    row-sum accumulation, writing fp16 factors
  - factor blocks PE-transposed to factorT [s, c]; mm2 is column-tiled
    over the two h-halves (strip0 -> h 0:512 into bank0 rows 0:64,
    strip1 -> h 512:1024 into bank1 rows 64:128), so the final
    normalize-evac reads a single PSUM region per op
  - row-sum normalization: micro PE transposes fold the per-half sums,
    DVE reciprocal, scale on evac, DMA out
"""

import numpy as np

import concourse.bass as bass
import concourse.mybir as mybir
import concourse.tile as tile
from concourse import bacc
from concourse.bass_utils import run_bass_kernel_spmd

B, S, H, C = 16, 4096, 1024, 64
NCORES = 8
BPC = B // NCORES          # batches per core
ST = 8                     # s-tiles per batch (512 rows each)
SQ = 4                     # 128-row subtiles per s-tile
HJ = H // 128              # h-chunks (8)

F32 = mybir.dt.float32
F16 = mybir.dt.float16
ADD = mybir.AluOpType.add
MIN = mybir.AluOpType.min
MAX = mybir.AluOpType.max
AXX = mybir.AxisListType.X
EXP = mybir.ActivationFunctionType.Exp
CPY = mybir.ActivationFunctionType.Copy

POS_BIG = 3.0e38


def build_nc():
    nc = bacc.Bacc("TRN2", target_bir_lowering=False, debug=False)
    hidden = nc.declare_dram_parameter("hidden", [BPC, S, H], F32, isOutput=False)
    qT = nc.declare_dram_parameter("qT", [128, HJ, C], F16, isOutput=False)
    ident = nc.declare_dram_parameter("ident", [128, 128], F16, isOutput=False)
    out = nc.declare_dram_parameter("out", [BPC, C, H], F32, isOutput=True)

    with tile.TileContext(nc) as tc:
        with (
            tc.tile_pool(name="const", bufs=1) as const_pool,
            tc.tile_pool(name="nat", bufs=16) as nat_pool,
            tc.tile_pool(name="hT", bufs=8) as hT_pool,
            tc.tile_pool(name="scores", bufs=2) as scores_pool,
            tc.tile_pool(name="expf", bufs=2) as exp_pool,
            tc.tile_pool(name="fT", bufs=2) as fT_pool,
            tc.tile_pool(name="stats", bufs=2) as stats_pool,
            tc.tile_pool(name="outp", bufs=2) as out_pool,
            tc.tile_pool(name="psT", bufs=2, space="PSUM") as psT_pool,
            tc.tile_pool(name="psS", bufs=2, space="PSUM") as psS_pool,
            tc.tile_pool(name="psF", bufs=2, space="PSUM") as psF_pool,
            tc.tile_pool(name="psR", bufs=2, space="PSUM") as psR_pool,
        ):
            ident_sb = const_pool.tile([128, 128], F16, tag="ident")
            nc.sync.dma_start(out=ident_sb[:], in_=ident[:])
            qT_sb = const_pool.tile([128, HJ, C], F16, tag="qT")
            nc.sync.dma_start(out=qT_sb[:], in_=qT[:])
            zeros_sb = const_pool.tile([64, 256], F32, tag="zeros")
            nc.vector.memset(zeros_sb[:], 0.0)

            nat_tiles = {}
            scores_tiles = {}
            exp_tiles = {}
            rs_tiles = {}
            rm_state = {}      # (b, half) -> running -(max) tile (chain head)
            rm_final = {}      # (b, half) -> final -(max) tile
            psR_tiles = {}

            def p1(b, st):
                """Load s-tile, transpose to hT, col-tiled mm1, evac + max."""
                nat_t = nat_pool.tile([128, SQ, H], F16, tag="nat")
                nat_tiles[(b, st)] = nat_t
                src = hidden[b, st * 512:(st + 1) * 512, :].rearrange(
                    "(q p) h -> p q h", p=128
                )
                nc.gpsimd.dma_start(out=nat_t[:], in_=src)

                if st == 0:
                    scores_tiles[b] = scores_pool.tile(
                        [64, S], F32, tag="scores", name="scores"
                    )
                scores_sb = scores_tiles[b]

                ps_sc = psS_pool.tile([128, 256], F32, tag="psS")
                for jp in range(4):
                    ps_t = psT_pool.tile([128, 1024], F16, tag="psT")
                    hT = hT_pool.tile([128, 1024], F16, tag="hT")
                    for ji in range(2):
                        j = jp * 2 + ji
                        for q in range(SQ):
                            nc.tensor.transpose(
                                ps_t[:, ji * 512 + q * 128:
                                     ji * 512 + (q + 1) * 128],
                                nat_t[:, q, j * 128:(j + 1) * 128],
                                ident_sb[:],
                            )
                    if jp == 2:
                        nc.scalar.copy(hT[:], ps_t[:])
                    else:
                        nc.vector.tensor_copy(hT[:], ps_t[:])
                    # col-tiled pairs: s-half 0 in strip (0,0), s-half 1 in
                    # strip (0,64); both share stationary qT[j]
                    for ji in range(2):
                        j = jp * 2 + ji
                        for sh in range(2):
                            nc.tensor.matmul(
                                ps_sc[sh * 64:(sh + 1) * 64, :],
                                qT_sb[:, j, :],
                                hT[:, ji * 512 + sh * 256:
                                   ji * 512 + (sh + 1) * 256],
                                start=(j == 0),
                                stop=(j == 7),
                                tile_position=(0, sh * 64),
                                skip_group_check=True,
                            )

                # evac strips into scores: DVE for strip 0, ScalarE for
                # strip 1; then fold this s-tile into the running row max
                nc.vector.tensor_copy(
                    scores_sb[:, st * 512:st * 512 + 256], ps_sc[0:64, :]
                )
                nc.scalar.copy(
                    scores_sb[:, st * 512 + 256:(st + 1) * 512],
                    ps_sc[64:128, :],
                )
                pm = stats_pool.tile([C, 1], F32, tag="pm", bufs=4)
                nc.vector.reduce_max(
                    pm[:], scores_sb[:, st * 512:(st + 1) * 512], axis=AXX
                )
                if st == 0:
                    rm_state[b] = pm
                else:
                    rmn = stats_pool.tile([C, 1], F32, tag="rm", bufs=8)
                    nc.vector.scalar_tensor_tensor(
                        out=rmn[:], in0=pm[:], scalar=0.0,
                        in1=rm_state[b][:], op0=ADD, op1=MAX,
                    )
                    rm_state[b] = rmn
                if st == ST - 1:
                    negmax = stats_pool.tile([C, 1], F32, tag="negmax")
                    nc.vector.tensor_scalar_mul(
                        negmax[:], rm_state[b][:], -1.0
                    )
                    rm_final[b] = negmax
                    exp_tiles[b] = exp_pool.tile([64, S], F16,
                                                 tag="expf", name="expf")
                    rs_tiles[b] = stats_pool.tile([C, ST], F32,
                                                  tag="rs", name="rs")
                    psR_tiles[b] = (
                        psR_pool.tile([128, 512], F32, tag="psR0",
                                      name="psR0", bufs=1),
                        psR_pool.tile([128, 512], F32, tag="psR1",
                                      name="psR1", bufs=1),
                    )

            def smp3(b, ci):
                """exp chunk ci (512 cols) + factorT + col-tiled mm2."""
                scores_sb = scores_tiles[b]
                exp_sb = exp_tiles[b]
                rs = rs_tiles[b]
                nc.scalar.activation(
                    exp_sb[:, ci * 512:(ci + 1) * 512],
                    scores_sb[:, ci * 512:(ci + 1) * 512],
                    EXP,
                    bias=rm_final[b][:],
                    scale=1.0,
                    accum_out=rs[:, ci:ci + 1],
                )
                ph = psR_tiles[b]
                idn = ident_sb[0:64, 0:64]
                ps_f = psF_pool.tile([128, 4 * C], F16, tag="psF")
                fTt = fT_pool.tile([128, 4 * C], F16, tag="fT")
                for ki in range(4):
                    nc.tensor.transpose(
                        ps_f[:, ki * C:(ki + 1) * C],
                        exp_sb[:, ci * 512 + ki * 128:
                               ci * 512 + (ki + 1) * 128],
                        idn,
                    )
                if ci % 2 == 0:
                    nc.scalar.copy(fTt[:], ps_f[:])
                else:
                    nc.vector.tensor_copy(fTt[:], ps_f[:])
                # col-tiled pairs: h-half 0 -> strip (0,0) bank0 rows 0:64,
                # h-half 1 -> strip (0,64) bank1 rows 64:128; both strips
                # share stationary fT[k]
                for ki in range(4):
                    k = ci * 4 + ki
                    natk = nat_tiles[(b, ci)]
                    fk = fTt[:, ki * C:(ki + 1) * C]
                    nc.tensor.matmul(
                        ph[0][0:64, :],
                        fk,
                        natk[:, ki, 0:512],
                        start=(k == 0),
                        stop=(k == 31),
                        tile_position=(0, 0),
                        skip_group_check=True,
                    )
                    nc.tensor.matmul(
                        ph[1][64:128, :],
                        fk,
                        natk[:, ki, 512:1024],
                        start=(k == 0),
                        stop=(k == 31),
                        tile_position=(0, 64),
                        skip_group_check=True,
                    )

            def finalize(b):
                """row-sum fold, reciprocal, scale on evac, DMA out."""
                rsum = stats_pool.tile([C, 1], F32, tag="rsum")
                nc.vector.reduce_sum(rsum[:], rs_tiles[b][:], axis=AXX)
                rinv = stats_pool.tile([C, 1], F32, tag="rinv")
                nc.vector.reciprocal(rinv[:], rsum[:])

                ph = psR_tiles[b]
                out_sb = out_pool.tile([C, H], F32, tag="out")
                nc.vector.tensor_scalar_mul(
                    out_sb[:, 0:512], ph[0][0:64, :], rinv[:]
                )
                nc.vector.tensor_scalar_mul(
                    out_sb[:, 512:1024], ph[1][64:128, :], rinv[:]
                )
                nc.sync.dma_start(out=out[b], in_=out_sb[:])

            # phase-interleaved schedule: batch 0 loads + mm1 first; its
            # softmax+mm2 chunks interleave with batch 1's s-tiles so the
            # PE never idles and the DMA queue stays saturated
            for st in range(ST):
                p1(0, st)
            p1(1, 0)
            smp3(0, 0); smp3(0, 1)
            p1(1, 1)
            smp3(0, 2); smp3(0, 3)
            p1(1, 2)
            smp3(0, 4); smp3(0, 5)
            p1(1, 3)
            smp3(0, 6); smp3(0, 7)
            finalize(0)
            p1(1, 4); p1(1, 5); p1(1, 6); p1(1, 7)
            for ci in range(ST):
                smp3(1, ci)
            finalize(1)

    nc.compile()
    return nc


_NC_CACHE = None


def _get_nc():
    global _NC_CACHE
    if _NC_CACHE is None:
        _NC_CACHE = build_nc()
    return _NC_CACHE


def kernel(hidden, querys):
    hidden = np.ascontiguousarray(np.asarray(hidden), dtype=np.float32)
    querys = np.ascontiguousarray(np.asarray(querys), dtype=np.float32)
    assert hidden.shape == (B, S, H) and querys.shape == (C, H)

    # qT[k, j, c] = querys[c, j*128 + k]  (h-chunk-major transposed layout)
    qT = np.ascontiguousarray(
        querys.T.reshape(HJ, 128, C).transpose(1, 0, 2)
    ).astype(np.float16)
    ident = np.eye(128, dtype=np.float16)

    nc = _get_nc()
    in_maps = [
        {
            "hidden": np.ascontiguousarray(hidden[i * BPC:(i + 1) * BPC]),
            "qT": qT,
            "ident": ident,
        }
        for i in range(NCORES)
    ]
    res = run_bass_kernel_spmd(nc, in_maps, core_ids=list(range(NCORES)))
    global LAST_RESULTS
    LAST_RESULTS = res
    outs = [np.asarray(res.results[i]["out"]).reshape(BPC, C * H)
            for i in range(NCORES)]
    return np.concatenate(outs, axis=0)


LAST_RESULTS = None
